# revision 81
# baseline (speedup 1.0000x reference)
"""Bahdanau additive attention on 8 Trainium2 NeuronCores.

Problem: B=32, S=1024, H=1024 fp32.
  U_h   = dec @ U_w.T                    [B, H]
  W_s   = enc @ W_w.T                    [B, S, H]
  att   = tanh(U_h[:,None,:] + W_s) @ v  [B, S]
  alpha = softmax(att, axis=1)
  ctx   = einsum('bs,bsh->bh', alpha, enc)

Sharding: data-parallel over B across 8 cores (4 batches per core),
W_w / v_w replicated. Host-side prep: casts / transposes / fp8
pair-interleaving, plus the tiny U_h projection (0.1% of FLOPs), so the
device kernel is a pure W_s-GEMM + softmax + context pipeline.

Per-core design:
  - PE warm-up: the HAM activity monitor clock-gates the PE to 1.2 GHz
    until ~3.4us of sustained activity. A run of dependency-free dummy
    matmuls issued at kernel start (while enc/W stream from HBM) lifts
    the gate so the real GEMM starts at 2.4 GHz. A couple more dummies
    bridge the b3 epilogue waits so the MID window never re-throttles.
  - The dominant W_s GEMM (1024^3 MACs x 4 batches) runs as a hybrid
    split-K: h-tiles 0-3 in fp8 e4m3 with DoubleRow perf mode and
    h-tiles 4-7 in bf16, accumulating into the same PSUM banks. The
    4/8 split keeps e4m3 quantization noise at rel~1.7e-2 (< 2e-2).
  - Iterations are emitted in groups of two with phase clustering
    [fp8(i),fp8(i+1)][bf16(i),bf16(i+1)] so the PE pays the fp8<->bf16
    array-mode switch half as often. 2 iters hold 4 PSUM banks (mm pool
    bufs=5) - 3 would deadlock.
  - ScalarE applies tanh with per-partition bias U_hT while evacuating
    PSUM; the v-matvec accumulates att on the PE (2 concurrent column
    groups per matvec), flushed in two clusters per batch (after groups
    1 and 3) to minimize col-tiling mode transitions.
  - Softmax skips the max-subtraction (att is bounded by ||v||_1).
  - Context for batches 0..2 runs OFF the PE: alpha broadcast by a
    one-row PE matmul, then DVE scalar_tensor_tensor fused
    multiply+accumulate against encT tiles already in SBUF; pipelined
    inside the next batch's groups.
  - The LAST batch's context uses a low-latency PE path: p = exp(att)
    is PE-transposed and multiplied against a natural-layout bf16 copy
    of that batch's enc, interleaved per s-half with the exp chunks;
    1/Z is folded into the PSUM evacuation.
  - DMA: per-queue rate scales with the per-partition line size, so
    everything moves as big-line transfers. enc8 leads on sync and W8
    on scalar (the two fp8-critical tensors deliver in parallel under
    either queue-boot order), the bf16 halves (encbf on sync, Wbf on
    scalar) follow, encc rides scalar behind them; enc streams on sync
    one batch ahead; tiny constants + outputs ride gpsimd.
"""

import numpy as np
import ml_dtypes
from contextlib import ExitStack

import concourse.bacc as bacc
import concourse.mybir as mybir
import concourse.tile as tile
from concourse.bass_utils import run_bass_kernel_spmd

N_CORES = 8
B = 32
B_L = B // N_CORES  # 4 batches per core
S = 1024
H = 1024
P = 128
NT = 8  # 1024 / 128 tiles
F32 = mybir.dt.float32
BF16 = mybir.dt.bfloat16
AF = mybir.ActivationFunctionType
ALU = mybir.AluOpType
BF = ml_dtypes.bfloat16
E4 = ml_dtypes.float8_e4m3fn
FP8 = mybir.dt.float8e4
DR = mybir.MatmulPerfMode.DoubleRow
N8 = 4          # h-tiles 0..3 computed in fp8 DoubleRow (pairs {0,1},{2,3})
NP = N8 // 2    # fp8 pair count
NBF = NT - N8   # bf16 h-tiles 4..7
# HAM un-throttles only after a COMPLETE free-running 4096-cycle window
# (3.41us at 1.2 GHz) of sustained activity; the window phase is arbitrary,
# so the warm-up must span TWO windows (6.8us) to guarantee a fire on
# every core regardless of phase. Once it fires mid-warm-up, the remaining
# dummies run at 2.4 GHz, so the guaranteed case barely delays real work.
N_WARM = 16


def _emit(tc):
    nc = tc.nc
    enc8_d = nc.dram_tensor("enc8", [B_L, P, NP, 2, S], FP8, kind="ExternalInput").ap()
    encbf_d = nc.dram_tensor("encbf", [B_L, P, NBF, S], BF16, kind="ExternalInput").ap()
    encc_d = nc.dram_tensor("encc", [B_L, P, N8, S], BF16, kind="ExternalInput").ap()
    W8_d = nc.dram_tensor("W8", [P, NT, NP, 2, P], FP8, kind="ExternalInput").ap()
    Wbf_d = nc.dram_tensor("Wbf", [P, NBF, NT, P], BF16, kind="ExternalInput").ap()
    UhT_d = nc.dram_tensor("UhT", [P, NT, B_L], F32, kind="ExternalInput").ap()
    vT_d = nc.dram_tensor("vT", [P, NT], BF16, kind="ExternalInput").ap()
    ident_d = nc.dram_tensor("ident", [P, P], F32, kind="ExternalInput").ap()
    encN3_d = nc.dram_tensor("encN3", [P, NT, H], BF16, kind="ExternalInput").ap()
    ctx_out = nc.dram_tensor("ctx", [B_L, H], F32, kind="ExternalOutput").ap()
    alpha_out = nc.dram_tensor("alpha", [B_L, S], F32, kind="ExternalOutput").ap()

    ctx = ExitStack()
    const = ctx.enter_context(tc.tile_pool(name="const", bufs=1))
    encTp = ctx.enter_context(tc.tile_pool(name="encT", bufs=3))
    thp = ctx.enter_context(tc.tile_pool(name="tanh", bufs=5))
    stgp = ctx.enter_context(tc.tile_pool(name="stg", bufs=1))
    psp = ctx.enter_context(tc.tile_pool(name="ps", bufs=1, space="PSUM"))

    # --- SBUF constants ---
    W8_sb = const.tile([P, NT, NP, 2, P], FP8)   # [p, i, jp, i2, oo] 4KiB/part
    Wbf_sb = const.tile([P, NBF, NT, P], BF16)   # [p, jj, i, oo]     8KiB/part
    U_hT = const.tile([P, NT, B_L], F32)         # host-computed U_h, transposed
    v_sb = const.tile([P, NT], BF16)
    identf = const.tile([P, P], F32)
    ones_sb = const.tile([1, P], BF16)
    junk_sb = const.tile([P, 512], BF16, name="junk")
    nc.vector.memset(ones_sb[0:1, :], 1.0)
    # gpsimd memset: its queue prologue finishes earliest, so the warm-up
    # matmuls (which read junk_sb) can start the moment the PE queue opens
    nc.gpsimd.memset(junk_sb[:, :], 0.5)

    # --- PE warm-up: lift the HAM clock gate while DMAs stream. Must be
    # full-width (K=M=128) matmuls: narrow ones don't register as activity ---
    def warm(n):
        wps = psp.tile([P, 512], F32, tag="mm", bufs=5, name="warm_ps")
        for _ in range(n):
            nc.tensor.matmul(
                wps[:, :], junk_sb[:, 0:P], junk_sb[:, :],
                start=True, stop=True,
            )

    warm(N_WARM)

    # --- prologue DMAs. Per-queue DMA rate scales with the per-partition
    # line size (4-8KB lines ~260-300GB/s, 2KB ~150, gpsimd SWDGE ~100).
    # Startup dovetail: W8+enc8 land ~11us feeding fp8 groups 0-1 (3.5us of
    # PE work) while Wbf (scalar, slack until 14) + encbf stream behind. ---
    enc8_cur = encTp.tile([P, NP, 2, S], FP8, tag="e8", name="enc8_0")
    encbf_cur = encTp.tile([P, NBF, S], BF16, tag="ebf", name="encbf_0")
    encc_cur = encTp.tile([P, N8, S], BF16, tag="ecc", name="encc_0")
    # Queue boot order varies ~2us run to run, so both queues carry
    # need-ordered streams: the two fp8-critical tensors (enc8, W8) lead
    # on separate queues; the bf16 halves follow on both.
    # sync: enc8 (4KB lines), then the bf16 enc halves
    nc.sync.dma_start(enc8_cur[:], enc8_d[0])
    nc.sync.dma_start(encbf_cur[:, 0:2, :], encbf_d[0, :, 0:2, :])
    nc.sync.dma_start(encbf_cur[:, 2:4, :], encbf_d[0, :, 2:4, :])
    # scalar: W8, Wbf halves (4KB lines), then the encc copies
    nc.scalar.dma_start(W8_sb[:], W8_d[:])
    nc.scalar.dma_start(Wbf_sb[:, 0:2], Wbf_d[:, 0:2])
    nc.scalar.dma_start(Wbf_sb[:, 2:4], Wbf_d[:, 2:4])
    nc.scalar.dma_start(encc_cur[:], encc_d[0])
    # gpsimd: tiny constants (outputs ride it later)
    nc.gpsimd.dma_start(U_hT[:], UhT_d[:])
    nc.gpsimd.dma_start(v_sb[:], vT_d[:])
    nc.gpsimd.dma_start(identf[:], ident_d[:])

    # --- staged context reduction for batch b (runs inside batch b+1) ---
    alpha_b16 = const.tile([1, S], BF16)
    pbc_sb = const.tile([P, S], BF16)
    ctxT = const.tile([P, NT], F32)
    encN3_sb = const.tile([P, NT, H], BF16)  # last batch, s on partitions
    fsc_v = const.tile([P, S], BF16, name="fsc_v")
    fsc_g = const.tile([P, S], BF16, name="fsc_g")

    def ctx_stage_bcast():
        """PE: broadcast alpha row across 128 partitions; DVE: evac to bf16."""
        pbc_ps = [
            psp.tile([P, 512], F32, tag="u", bufs=2, name=f"pbc{c}") for c in range(2)
        ]
        for c in range(2):
            nc.tensor.matmul(
                pbc_ps[c][:],
                ones_sb[0:1, :],
                alpha_b16[0:1, 512 * c : 512 * (c + 1)],
                start=True,
                stop=True,
            )
        for c in range(2):
            nc.vector.tensor_copy(pbc_sb[:, 512 * c : 512 * (c + 1)], pbc_ps[c][:])

    def ctx_stage_fused(encc_b, encbf_b):
        """DVE: ctxT[p,j] = sum_s enc[b,s,128j+p] * alpha[b,s].
        (scalar_tensor_tensor is DVE-only: Pool engine fails codegen.)"""
        for j in range(NT):
            scratch = fsc_v if j % 2 == 0 else fsc_g
            in0 = encc_b[:, j, :] if j < N8 else encbf_b[:, j - N8, :]
            nc.vector.scalar_tensor_tensor(
                scratch[:],
                in0,
                1.0,
                pbc_sb[:],
                ALU.mult,
                ALU.mult,
                accum_out=ctxT[:, j : j + 1],
            )

    def ctx_stage_out(b):
        """Scale ctx^T by 1/Z (alpha is unnormalized until here) and DMA it
        out with a strided (transposing) access pattern: slow scattered 4B
        writes, but the row has ~30us of deadline slack and this keeps the
        PE transpose + full-row normalization out of the pipeline."""
        ctx_stg = stgp.tile([P, NT], F32, tag="ctxstg")
        nc.vector.tensor_copy(ctx_stg[:], ctxT[:])
        nc.gpsimd.dma_start(
            ctx_out[b].rearrange("(t p) -> p t", p=P), ctx_stg[:]
        )

    # att row layout: three concurrent column strips of the PE array
    # (quadrant 3 is unusable), rows 0/32/64 of the att bank
    STRIPS = [(0, 0, 342), (1, 342, 342), (2, 684, 340)]

    def emit_matvec(ip, th, att_ps, first, last):
        for s, off, n in STRIPS:
            nc.tensor.matmul(
                att_ps[32 * s : 32 * s + 1, 0:n],
                v_sb[:, ip : ip + 1],
                th[:, off : off + n],
                start=first,
                stop=last,
                tile_position=(0, 32 * s),
            )

    def emit_fp8(i, ps):
        for jp in range(NP):
            lhsT8 = W8_sb[:, i, jp, :, :]
            for c in range(2):
                for q in range(2):
                    s0 = 512 * c + 256 * q
                    # start only on the bank's first write: start=True
                    # zeroes the full 2KiB PSUM row (ZERO_REGION).
                    # N=256 q-chunks: DoubleRow only sustains 2 elem/cycle
                    # up to 512 moving elements per matmul.
                    nc.tensor.matmul(
                        ps[c][:, 256 * q : 256 * (q + 1)],
                        lhsT8,
                        enc8_cur[:, jp, :, s0 : s0 + 256],
                        start=(jp == 0 and q == 0),
                        stop=False,
                        perf_mode=DR,
                        skip_group_check=True,
                    )

    def emit_bf16(i, ps, warm_mid=False):
        for jj in range(NBF):
            if warm_mid and jj == 2:
                warm(4)  # b0 g0: bridge the encbf jj2/jj3 DMA wait
            lhsT = Wbf_sb[:, jj, i, :]
            for c in range(2):
                nc.tensor.matmul(
                    ps[c][:],
                    lhsT,
                    encbf_cur[:, jj, c * 512 : (c + 1) * 512],
                    start=False,
                    stop=(jj == NBF - 1),
                    skip_group_check=True,
                )

    def emit_tanh(i, b, ps, th):
        for c in range(2):
            nc.scalar.activation(
                th[:, c * 512 : (c + 1) * 512],
                ps[c][:],
                AF.Tanh,
                bias=U_hT[:, i, b : b + 1],
                scale=1.0,
            )

    enc_prev = None  # (encc, encbf) tiles of batch b-1 for the ctx fused stage
    for b in range(B_L):
        enc_next = None
        if b + 1 < B_L:
            enc8_next = encTp.tile([P, NP, 2, S], FP8, tag="e8", name=f"enc8_{b+1}")
            encbf_next = encTp.tile([P, NBF, S], BF16, tag="ebf", name=f"encbf_{b+1}")
            nc.sync.dma_start(enc8_next[:], enc8_d[b + 1])
            nc.sync.dma_start(encbf_next[:], encbf_d[b + 1])
            encc_next = None
            if b + 2 < B_L:
                # the last batch's ctx runs on the PE from encN3: no encc needed
                encc_next = encTp.tile([P, N8, S], BF16, tag="ecc", name=f"encc_{b+1}")
                nc.scalar.dma_start(encc_next[:], encc_d[b + 1])
            enc_next = (enc8_next, encbf_next, encc_next)
        else:
            nc.sync.dma_start(encN3_sb[:], encN3_d[:])

        att_ps = psp.tile([P, 512], F32, tag="att", name="att_ps")
        th_of = {}
        next_mv = 0

        def flush_mv(ip_max):
            nonlocal next_mv
            for ip in range(next_mv, ip_max + 1):
                emit_matvec(ip, th_of.pop(ip), att_ps, ip == 0, False)
            next_mv = ip_max + 1

        for g in range(NT // 2):
            i0, i1 = 2 * g, 2 * g + 1
            ps_of = {}
            for i in (i0, i1):
                ps = [
                    psp.tile([P, 512], F32, tag="mm", bufs=5, name=f"mm_ps{c2}")
                    for c2 in range(2)
                ]
                ps_of[i] = ps
                emit_fp8(i, ps)
            last_grp = b == B_L - 1 and g == NT // 2 - 1
            if b == 0 and g == 0:
                warm(5)  # bridge the encbf-jj01 DMA wait; keep HAM open
            for i in (i0, i1):
                ps = ps_of[i]
                emit_bf16(i, ps, warm_mid=(b == 0 and g == 0 and i == i0))
                th = thp.tile([P, S], BF16, tag="tanh")
                if last_grp and i == i1:
                    # b3 tail: flush pending matvecs (they overlap tanh(7)),
                    # then per-chunk tanh -> matvec so ScalarE/PE overlap
                    flush_mv(NT - 2)
                    for c in range(2):
                        nc.scalar.activation(
                            th[:, c * 512 : (c + 1) * 512],
                            ps[c][:],
                            AF.Tanh,
                            bias=U_hT[:, i, b : b + 1],
                            scale=1.0,
                        )
                        if c == 1:
                            warm(1)  # hide tanh(7) c1 ScalarE latency
                        for s, off, n in (STRIPS[:1] if c == 0 else STRIPS[1:]):
                            nc.tensor.matmul(
                                att_ps[32 * s : 32 * s + 1, 0:n],
                                v_sb[:, i : i + 1],
                                th[:, off : off + n],
                                start=False,
                                stop=True,
                                tile_position=(0, 32 * s),
                            )
                else:
                    emit_tanh(i, b, ps, th)
                    th_of[i] = th
            # matvecs flush in two clusters per batch (fewer PE mode switches);
            # the flush upper bound lags one i-tile so tanh is always ready
            if last_grp:
                pass  # matvecs already emitted inline above
            elif g % 2 == 1:
                flush_mv(i1 - 1)
            # pipelined ctx stages for batch b-1 (bf16-mode PE ops, placed
            # adjacent to the matvec block to avoid extra mode switches)
            if b > 0:
                if g == 0:
                    ctx_stage_bcast()
                elif g == 1:
                    ctx_stage_fused(enc_prev[2], enc_prev[1])
                elif g == 2:
                    ctx_stage_out(b - 1)

        if b < B_L - 1:
            # trailing matvec for i=7: strip 0 only needs tanh(7) chunk c0;
            # a single warm MM hides the deterministic ~270ns ScalarE
            # latency before chunk c1 (strips 1-2) lands
            th7 = th_of.pop(NT - 1)
            for k, (s, off, n) in enumerate(STRIPS):
                if k == 1:
                    warm(1)
                nc.tensor.matmul(
                    att_ps[32 * s : 32 * s + 1, 0:n],
                    v_sb[:, NT - 1 : NT],
                    th7[:, off : off + n],
                    start=False,
                    stop=True,
                    tile_position=(0, 32 * s),
                )

        # --- per-batch epilogue: exp straight from PSUM (no max needed; the
        # att bank is free long before the next batch's first matvec). The
        # softmax normalization happens on the HOST: the device ships raw
        # exp rows and only folds 1/Z into the context reductions. ---
        if b < B_L - 1:
            exp_stg = stgp.tile([1, S], F32, tag="expstg")
            ssum2 = stgp.tile([1, 3], F32, tag="ssum2")
            for s, off, n in STRIPS:
                nc.scalar.activation(
                    exp_stg[0:1, off : off + n],
                    att_ps[32 * s : 32 * s + 1, 0:n],
                    AF.Exp,
                    accum_out=ssum2[0:1, s : s + 1],
                )
            ssum = stgp.tile([1, 1], F32, tag="ssum")
            nc.vector.reduce_sum(ssum[:], ssum2[:], axis=mybir.AxisListType.X)
            srec = stgp.tile([1, 1], F32, tag="srec")
            nc.vector.reciprocal(srec[:], ssum[:])
            nc.gpsimd.dma_start(alpha_out[b : b + 1, :], exp_stg[0:1, :])
            # normalized bf16 alpha for the next batch's DVE context path
            nc.vector.tensor_scalar_mul(alpha_b16[0:1, :], exp_stg[0:1, :], srec[:])

            enc_prev = (enc8_cur, encbf_cur, encc_cur)
            enc8_cur, encbf_cur, encc_cur = enc_next
            continue

        # --- final batch: interleaved softmax + PE context chain ---
        # exp straight from PSUM in quarter chunks so the p-transposes can
        # start ~370ns after the last matvec; 1/Z folded into the evac
        exp_stg = stgp.tile([1, S], F32, tag="expstg")
        ssum4 = stgp.tile([1, 3], F32, tag="ssum4")
        for s, off, n in STRIPS:
            nc.scalar.activation(
                exp_stg[0:1, off : off + n],
                att_ps[32 * s : 32 * s + 1, 0:n],
                AF.Exp,
                accum_out=ssum4[0:1, s : s + 1],
            )
        warm(4)  # bridge the exp wait; keep the HAM gate open
        psk = psp.tile([P, NT], F32, tag="mm", bufs=5, name="psk")
        pT_sb = stgp.tile([P, NT], BF16, tag="pT")
        # two PSUM tiles (one per column strip): cross-engine readers of a
        # single PSUM tile serialize, and the evac runs on ScalarE + DVE
        ps3t = [
            psp.tile([P, 512], F32, tag="u", bufs=2, name=f"ps3{c}")
            for c in range(2)
        ]
        # all transposes first, grouped by which exp strip covers them, then
        # the context matvecs run back-to-back with no mid-chain exp wait
        for k0, k1 in ((0, 2), (2, 5), (5, 8)):
            for k in range(k0, k1):
                nc.tensor.transpose(
                    psk[:, k : k + 1],
                    exp_stg[0:1, k * P : (k + 1) * P],
                    identf[0:1, 0:1],
                )
            nc.vector.tensor_copy(pT_sb[:, k0:k1], psk[:, k0:k1])
        for k in range(NT):
            lhsT = pT_sb[:, k : k + 1]
            for c in range(2):
                nc.tensor.matmul(
                    ps3t[c][32 * c : 32 * c + 1, :],
                    lhsT,
                    encN3_sb[:, k, c * 512 : (c + 1) * 512],
                    start=(k == 0),
                    stop=(k == NT - 1),
                    tile_position=(0, 32 * c),
                )
        ssum = stgp.tile([1, 1], F32, tag="ssum")
        nc.vector.reduce_sum(ssum[:], ssum4[:], axis=mybir.AxisListType.X)
        srec = stgp.tile([1, 1], F32, tag="srec")
        nc.vector.reciprocal(srec[:], ssum[:])
        nc.scalar.dma_start(alpha_out[b : b + 1, :], exp_stg[0:1, :])
        # 1/Z-scaled PSUM evac into TWO tiles (a shared tile serializes the
        # cross-engine writers) with parallel DMAs on separate queues
        ctx3a = stgp.tile([1, 512], F32, tag="ctx3a")
        ctx3b = stgp.tile([1, 512], F32, tag="ctx3b")
        nc.scalar.activation(
            ctx3a[0:1, :], ps3t[0][0:1, :], AF.Copy, scale=srec[0:1, 0:1]
        )
        nc.vector.tensor_scalar_mul(ctx3b[0:1, :], ps3t[1][32:33, :], srec[:])
        nc.sync.dma_start(ctx_out[B_L - 1 : B_L, 0:512], ctx3a[0:1, :])
        nc.scalar.dma_start(ctx_out[B_L - 1 : B_L, 512:1024], ctx3b[0:1, :])
    ctx.close()


_CACHED = None


def _build():
    global _CACHED
    if _CACHED is None:
        nc = bacc.Bacc("TRN2", target_bir_lowering=False, debug=False)
        with tile.TileContext(nc) as tc:
            _emit(tc)
        nc.compile()
        _CACHED = nc
    return _CACHED


def make_in_maps(decoder_hidden, encoder_outputs, U_w, W_w, v_w):
    """Host-side layout prep: casts, pre-transposes, U_h projection."""
    dec = np.asarray(decoder_hidden, dtype=np.float32)
    enc = np.asarray(encoder_outputs, dtype=np.float32)
    U = np.asarray(U_w, dtype=np.float32)
    W = np.asarray(W_w, dtype=np.float32)
    v = np.asarray(v_w, dtype=np.float32)

    # U_h[b, o] = sum_h dec[b, h] U[o, h]  (exact f32, matches reference)
    U_h = dec @ U.T

    # W8[p, i, jp, i2, oo] = fp8(W[128i+oo, 256jp+128i2+p]), h in [0, 512)
    W8 = np.ascontiguousarray(
        W[:, : 128 * N8].reshape(NT, P, NP, 2, P).transpose(4, 0, 2, 3, 1).astype(E4)
    )
    # Wbf[p, jj, i, oo] = W[128i+oo, 128(N8+jj)+p]
    Wbf = np.ascontiguousarray(
        W[:, 128 * N8 :].reshape(NT, P, NBF, P).transpose(3, 2, 0, 1).astype(BF)
    )
    ident = np.eye(P, dtype=np.float32)
    vT = np.ascontiguousarray(v.reshape(NT, P).T.astype(BF))

    in_maps = []
    for c in range(N_CORES):
        sl = slice(c * B_L, (c + 1) * B_L)
        enc_sl = enc[sl]  # [B_L, S, H]
        enc_t = enc_sl.transpose(0, 2, 1)  # [b, h, s]
        # enc8[b, p, jp, i2, s] = fp8(enc[b, s, 256jp+128i2+p]), h in [0, 512)
        enc8 = np.ascontiguousarray(
            enc_t[:, : 128 * N8].reshape(B_L, NP, 2, P, S).transpose(0, 3, 1, 2, 4).astype(E4)
        )
        # encbf[b, p, jj, s] = enc[b, s, 128(N8+jj)+p]
        encbf = np.ascontiguousarray(
            enc_t[:, 128 * N8 :].reshape(B_L, NBF, P, S).transpose(0, 2, 1, 3).astype(BF)
        )
        # encc[b, p, j, s] = enc[b, s, 128j+p] for j < N8 (bf16 ctx copy)
        encc = np.ascontiguousarray(
            enc_t[:, : 128 * N8].reshape(B_L, N8, P, S).transpose(0, 2, 1, 3).astype(BF)
        )
        # encN3[p, k, h] = enc[B_L-1, 128k+p, h] (last batch, natural layout)
        encN3 = np.ascontiguousarray(
            enc_sl[B_L - 1].reshape(NT, P, H).transpose(1, 0, 2).astype(BF)
        )
        # UhT[p, j, b] = U_h[b, 128j+p]
        UhT = np.ascontiguousarray(
            U_h[sl].reshape(B_L, NT, P).transpose(2, 1, 0).astype(np.float32)
        )
        in_maps.append(
            {
                "enc8": enc8,
                "encbf": encbf,
                "encc": encc,
                "W8": W8,
                "Wbf": Wbf,
                "UhT": UhT,
                "vT": vT,
                "ident": ident,
                "encN3": encN3,
            }
        )
    return in_maps


def kernel(
    decoder_hidden: np.ndarray,
    encoder_outputs: np.ndarray,
    U_w: np.ndarray,
    W_w: np.ndarray,
    v_w: np.ndarray,
):
    nc = _build()
    in_maps = make_in_maps(decoder_hidden, encoder_outputs, U_w, W_w, v_w)
    res = run_bass_kernel_spmd(nc, in_maps, core_ids=list(range(N_CORES)))
    context = np.concatenate([res.results[c]["ctx"] for c in range(N_CORES)], axis=0)
    # the device ships unnormalized exp(att) rows; normalize here
    p = np.concatenate([res.results[c]["alpha"] for c in range(N_CORES)], axis=0)
    alpha = p / p.sum(axis=1, keepdims=True)
    return (context.astype(np.float32), alpha.astype(np.float32))


# revision 83
# speedup vs baseline: 1.0166x; 1.0166x over previous
"""Bahdanau additive attention on 8 Trainium2 NeuronCores.

Problem: B=32, S=1024, H=1024 fp32.
  U_h   = dec @ U_w.T                    [B, H]
  W_s   = enc @ W_w.T                    [B, S, H]
  att   = tanh(U_h[:,None,:] + W_s) @ v  [B, S]
  alpha = softmax(att, axis=1)
  ctx   = einsum('bs,bsh->bh', alpha, enc)

Sharding: data-parallel over B across 8 cores (4 batches per core),
W_w / v_w replicated. Host-side prep: casts / transposes / fp8
pair-interleaving, plus the tiny U_h projection (0.1% of FLOPs), so the
device kernel is a pure W_s-GEMM + softmax + context pipeline.

Per-core design:
  - PE warm-up: the HAM activity monitor clock-gates the PE to 1.2 GHz
    until ~3.4us of sustained activity. A run of dependency-free dummy
    matmuls issued at kernel start (while enc/W stream from HBM) lifts
    the gate so the real GEMM starts at 2.4 GHz. A couple more dummies
    bridge the b3 epilogue waits so the MID window never re-throttles.
  - The dominant W_s GEMM (1024^3 MACs x 4 batches) runs as a hybrid
    split-K: h-tiles 0-3 in fp8 e4m3 with DoubleRow perf mode and
    h-tiles 4-7 in bf16, accumulating into the same PSUM banks. The
    4/8 split keeps e4m3 quantization noise at rel~1.7e-2 (< 2e-2).
  - Iterations are emitted in groups of two with phase clustering
    [fp8(i),fp8(i+1)][bf16(i),bf16(i+1)] so the PE pays the fp8<->bf16
    array-mode switch half as often. 2 iters hold 4 PSUM banks (mm pool
    bufs=5) - 3 would deadlock.
  - ScalarE applies tanh with per-partition bias U_hT while evacuating
    PSUM; the v-matvec accumulates att on the PE (2 concurrent column
    groups per matvec), flushed in two clusters per batch (after groups
    1 and 3) to minimize col-tiling mode transitions.
  - Softmax skips the max-subtraction (att is bounded by ||v||_1).
  - Context for batches 0..2 runs OFF the PE: alpha broadcast by a
    one-row PE matmul, then DVE scalar_tensor_tensor fused
    multiply+accumulate against encT tiles already in SBUF; pipelined
    inside the next batch's groups.
  - The LAST batch's context uses a low-latency PE path: p = exp(att)
    is PE-transposed and multiplied against a natural-layout bf16 copy
    of that batch's enc, interleaved per s-half with the exp chunks;
    1/Z is folded into the PSUM evacuation.
  - DMA: per-queue rate scales with the per-partition line size, so
    everything moves as big-line transfers. enc8 leads on sync and W8
    on scalar (the two fp8-critical tensors deliver in parallel under
    either queue-boot order), the bf16 halves (encbf on sync, Wbf on
    scalar) follow, encc rides scalar behind them; enc streams on sync
    one batch ahead; tiny constants + outputs ride gpsimd.
"""

import numpy as np
import ml_dtypes
from contextlib import ExitStack

import concourse.bacc as bacc
import concourse.mybir as mybir
import concourse.tile as tile
from concourse.bass_utils import run_bass_kernel_spmd

N_CORES = 8
B = 32
B_L = B // N_CORES  # 4 batches per core
S = 1024
H = 1024
P = 128
NT = 8  # 1024 / 128 tiles
F32 = mybir.dt.float32
BF16 = mybir.dt.bfloat16
AF = mybir.ActivationFunctionType
ALU = mybir.AluOpType
BF = ml_dtypes.bfloat16
E4 = ml_dtypes.float8_e4m3fn
FP8 = mybir.dt.float8e4
DR = mybir.MatmulPerfMode.DoubleRow
N8 = 4          # h-tiles 0..3 computed in fp8 DoubleRow (pairs {0,1},{2,3})
NP = N8 // 2    # fp8 pair count
NBF = NT - N8   # bf16 h-tiles 4..7
# HAM un-throttles only after a COMPLETE free-running 4096-cycle window
# (3.41us at 1.2 GHz) of sustained activity; the window phase is arbitrary,
# so the warm-up must span TWO windows (6.8us) to guarantee a fire on
# every core regardless of phase. Once it fires mid-warm-up, the remaining
# dummies run at 2.4 GHz, so the guaranteed case barely delays real work.
N_WARM = 16


def _emit(tc):
    nc = tc.nc
    enc8_d = nc.dram_tensor("enc8", [B_L, P, NP, 2, S], FP8, kind="ExternalInput").ap()
    encbf_d = nc.dram_tensor("encbf", [B_L, P, NBF, S], BF16, kind="ExternalInput").ap()
    encc_d = nc.dram_tensor("encc", [B_L, P, N8, S], BF16, kind="ExternalInput").ap()
    W8_d = nc.dram_tensor("W8", [P, NT, NP, 2, P], FP8, kind="ExternalInput").ap()
    Wbf_d = nc.dram_tensor("Wbf", [P, NBF, NT, P], BF16, kind="ExternalInput").ap()
    UhT_d = nc.dram_tensor("UhT", [P, NT, B_L], F32, kind="ExternalInput").ap()
    vT_d = nc.dram_tensor("vT", [P, NT], BF16, kind="ExternalInput").ap()
    ident_d = nc.dram_tensor("ident", [P, P], F32, kind="ExternalInput").ap()
    encN3_d = nc.dram_tensor("encN3", [P, NT, H], BF16, kind="ExternalInput").ap()
    ctx_out = nc.dram_tensor("ctx", [B_L, H], F32, kind="ExternalOutput").ap()
    alpha_out = nc.dram_tensor("alpha", [B_L, S], F32, kind="ExternalOutput").ap()

    ctx = ExitStack()
    const = ctx.enter_context(tc.tile_pool(name="const", bufs=1))
    encTp = ctx.enter_context(tc.tile_pool(name="encT", bufs=3))
    thp = ctx.enter_context(tc.tile_pool(name="tanh", bufs=5))
    stgp = ctx.enter_context(tc.tile_pool(name="stg", bufs=1))
    psp = ctx.enter_context(tc.tile_pool(name="ps", bufs=1, space="PSUM"))

    # --- SBUF constants ---
    W8_sb = const.tile([P, NT, NP, 2, P], FP8)   # [p, i, jp, i2, oo] 4KiB/part
    Wbf_sb = const.tile([P, NBF, NT, P], BF16)   # [p, jj, i, oo]     8KiB/part
    U_hT = const.tile([P, NT, B_L], F32)         # host-computed U_h, transposed
    v_sb = const.tile([P, NT], BF16)
    identf = const.tile([P, P], F32)
    ones_sb = const.tile([1, P], BF16)
    junk_sb = const.tile([P, 512], BF16, name="junk")
    nc.vector.memset(ones_sb[0:1, :], 1.0)
    # gpsimd memset: its queue prologue finishes earliest, so the warm-up
    # matmuls (which read junk_sb) can start the moment the PE queue opens
    nc.gpsimd.memset(junk_sb[:, :], 0.5)

    # --- PE warm-up: lift the HAM clock gate while DMAs stream. Must be
    # full-width (K=M=128) matmuls: narrow ones don't register as activity ---
    def warm(n):
        wps = psp.tile([P, 512], F32, tag="mm", bufs=5, name="warm_ps")
        for _ in range(n):
            nc.tensor.matmul(
                wps[:, :], junk_sb[:, 0:P], junk_sb[:, :],
                start=True, stop=True,
            )

    warm(N_WARM)

    # --- prologue DMAs. Per-queue DMA rate scales with the per-partition
    # line size (4-8KB lines ~260-300GB/s, 2KB ~150, gpsimd SWDGE ~100).
    # Startup dovetail: W8+enc8 land ~11us feeding fp8 groups 0-1 (3.5us of
    # PE work) while Wbf (scalar, slack until 14) + encbf stream behind. ---
    enc8_cur = encTp.tile([P, NP, 2, S], FP8, tag="e8", name="enc8_0")
    encbf_cur = encTp.tile([P, NBF, S], BF16, tag="ebf", name="encbf_0")
    encc_cur = encTp.tile([P, N8, S], BF16, tag="ecc", name="encc_0")
    # Queue boot order varies ~2us run to run, so both queues carry
    # need-ordered streams: the two fp8-critical tensors (enc8, W8) lead
    # on separate queues; the bf16 halves follow on both.
    # sync: enc8 (4KB lines), then the bf16 enc halves
    nc.sync.dma_start(enc8_cur[:], enc8_d[0])
    nc.sync.dma_start(encbf_cur[:, 0:2, :], encbf_d[0, :, 0:2, :])
    nc.sync.dma_start(encbf_cur[:, 2:4, :], encbf_d[0, :, 2:4, :])
    # scalar: W8, Wbf halves (4KB lines), then the encc copies
    nc.scalar.dma_start(W8_sb[:], W8_d[:])
    nc.scalar.dma_start(Wbf_sb[:, 0:2], Wbf_d[:, 0:2])
    nc.scalar.dma_start(Wbf_sb[:, 2:4], Wbf_d[:, 2:4])
    nc.scalar.dma_start(encc_cur[:], encc_d[0])
    # gpsimd: tiny constants (outputs ride it later)
    nc.gpsimd.dma_start(U_hT[:], UhT_d[:])
    nc.gpsimd.dma_start(v_sb[:], vT_d[:])
    nc.gpsimd.dma_start(identf[:], ident_d[:])

    # --- staged context reduction for batch b (runs inside batch b+1) ---
    alpha_b16 = const.tile([1, S], BF16)
    pbc_sb = const.tile([P, S], BF16)
    ctxT = const.tile([P, NT], F32)
    encN3_sb = const.tile([P, NT, H], BF16)  # last batch, s on partitions
    fsc_v = const.tile([P, S], BF16, name="fsc_v")
    fsc_g = const.tile([P, S], BF16, name="fsc_g")

    def ctx_stage_bcast():
        """PE: broadcast alpha row across 128 partitions; DVE: evac to bf16."""
        pbc_ps = [
            psp.tile([P, 512], F32, tag="u", bufs=2, name=f"pbc{c}") for c in range(2)
        ]
        for c in range(2):
            nc.tensor.matmul(
                pbc_ps[c][:],
                ones_sb[0:1, :],
                alpha_b16[0:1, 512 * c : 512 * (c + 1)],
                start=True,
                stop=True,
            )
        for c in range(2):
            nc.vector.tensor_copy(pbc_sb[:, 512 * c : 512 * (c + 1)], pbc_ps[c][:])

    def ctx_stage_fused(encc_b, encbf_b):
        """DVE: ctxT[p,j] = sum_s enc[b,s,128j+p] * alpha[b,s].
        (scalar_tensor_tensor is DVE-only: Pool engine fails codegen.)"""
        for j in range(NT):
            scratch = fsc_v if j % 2 == 0 else fsc_g
            in0 = encc_b[:, j, :] if j < N8 else encbf_b[:, j - N8, :]
            nc.vector.scalar_tensor_tensor(
                scratch[:],
                in0,
                1.0,
                pbc_sb[:],
                ALU.mult,
                ALU.mult,
                accum_out=ctxT[:, j : j + 1],
            )

    def ctx_stage_out(b):
        """Scale ctx^T by 1/Z (alpha is unnormalized until here) and DMA it
        out with a strided (transposing) access pattern: slow scattered 4B
        writes, but the row has ~30us of deadline slack and this keeps the
        PE transpose + full-row normalization out of the pipeline."""
        ctx_stg = stgp.tile([P, NT], F32, tag="ctxstg")
        nc.vector.tensor_copy(ctx_stg[:], ctxT[:])
        nc.gpsimd.dma_start(
            ctx_out[b].rearrange("(t p) -> p t", p=P), ctx_stg[:]
        )

    # att row layout: three concurrent column strips of the PE array
    # (quadrant 3 is unusable), rows 0/32/64 of the att bank
    STRIPS = [(0, 0, 342), (1, 342, 342), (2, 684, 340)]

    def emit_matvec(ip, th, att_ps, first, last):
        for s, off, n in STRIPS:
            nc.tensor.matmul(
                att_ps[32 * s : 32 * s + 1, 0:n],
                v_sb[:, ip : ip + 1],
                th[:, off : off + n],
                start=first,
                stop=last,
                tile_position=(0, 32 * s),
            )

    def emit_fp8(i, ps):
        for jp in range(NP):
            lhsT8 = W8_sb[:, i, jp, :, :]
            for c in range(2):
                for q in range(2):
                    s0 = 512 * c + 256 * q
                    # start only on the bank's first write: start=True
                    # zeroes the full 2KiB PSUM row (ZERO_REGION).
                    # N=256 q-chunks: DoubleRow only sustains 2 elem/cycle
                    # up to 512 moving elements per matmul.
                    nc.tensor.matmul(
                        ps[c][:, 256 * q : 256 * (q + 1)],
                        lhsT8,
                        enc8_cur[:, jp, :, s0 : s0 + 256],
                        start=(jp == 0 and q == 0),
                        stop=False,
                        perf_mode=DR,
                        skip_group_check=True,
                    )

    def emit_bf16(i, ps, warm_mid=False):
        for jj in range(NBF):
            if warm_mid and jj == 2:
                warm(2)  # b0 g0: bridge the encbf jj2/jj3 DMA wait
            lhsT = Wbf_sb[:, jj, i, :]
            for c in range(2):
                nc.tensor.matmul(
                    ps[c][:],
                    lhsT,
                    encbf_cur[:, jj, c * 512 : (c + 1) * 512],
                    start=False,
                    stop=(jj == NBF - 1),
                    skip_group_check=True,
                )

    def emit_tanh(i, b, ps, th):
        for c in range(2):
            nc.scalar.activation(
                th[:, c * 512 : (c + 1) * 512],
                ps[c][:],
                AF.Tanh,
                bias=U_hT[:, i, b : b + 1],
                scale=1.0,
            )

    enc_prev = None  # (encc, encbf) tiles of batch b-1 for the ctx fused stage
    for b in range(B_L):
        enc_next = None
        if b + 1 < B_L:
            enc8_next = encTp.tile([P, NP, 2, S], FP8, tag="e8", name=f"enc8_{b+1}")
            encbf_next = encTp.tile([P, NBF, S], BF16, tag="ebf", name=f"encbf_{b+1}")
            nc.sync.dma_start(enc8_next[:], enc8_d[b + 1])
            nc.sync.dma_start(encbf_next[:], encbf_d[b + 1])
            encc_next = None
            if b + 2 < B_L:
                # the last batch's ctx runs on the PE from encN3: no encc needed
                encc_next = encTp.tile([P, N8, S], BF16, tag="ecc", name=f"encc_{b+1}")
                nc.scalar.dma_start(encc_next[:], encc_d[b + 1])
            enc_next = (enc8_next, encbf_next, encc_next)
        else:
            nc.sync.dma_start(encN3_sb[:], encN3_d[:])

        att_ps = psp.tile([P, 512], F32, tag="att", name="att_ps")
        th_of = {}
        next_mv = 0

        def flush_mv(ip_max):
            nonlocal next_mv
            for ip in range(next_mv, ip_max + 1):
                emit_matvec(ip, th_of.pop(ip), att_ps, ip == 0, False)
            next_mv = ip_max + 1

        for g in range(NT // 2):
            i0, i1 = 2 * g, 2 * g + 1
            ps_of = {}
            for i in (i0, i1):
                ps = [
                    psp.tile([P, 512], F32, tag="mm", bufs=5, name=f"mm_ps{c2}")
                    for c2 in range(2)
                ]
                ps_of[i] = ps
                emit_fp8(i, ps)
            last_grp = b == B_L - 1 and g == NT // 2 - 1
            if b == 0 and g == 0:
                warm(3)  # bridge the encbf-jj01 DMA wait; keep HAM open
            for i in (i0, i1):
                ps = ps_of[i]
                emit_bf16(i, ps, warm_mid=(b == 0 and g == 0 and i == i0))
                th = thp.tile([P, S], BF16, tag="tanh")
                if last_grp and i == i1:
                    # b3 tail: flush pending matvecs (they overlap tanh(7)),
                    # then per-chunk tanh -> matvec so ScalarE/PE overlap
                    flush_mv(NT - 2)
                    for c in range(2):
                        nc.scalar.activation(
                            th[:, c * 512 : (c + 1) * 512],
                            ps[c][:],
                            AF.Tanh,
                            bias=U_hT[:, i, b : b + 1],
                            scale=1.0,
                        )
                        if c == 1:
                            warm(1)  # hide tanh(7) c1 ScalarE latency
                        for s, off, n in (STRIPS[:1] if c == 0 else STRIPS[1:]):
                            nc.tensor.matmul(
                                att_ps[32 * s : 32 * s + 1, 0:n],
                                v_sb[:, i : i + 1],
                                th[:, off : off + n],
                                start=False,
                                stop=True,
                                tile_position=(0, 32 * s),
                            )
                else:
                    emit_tanh(i, b, ps, th)
                    th_of[i] = th
            # matvecs flush in two clusters per batch (fewer PE mode switches);
            # the flush upper bound lags one i-tile so tanh is always ready
            if last_grp:
                pass  # matvecs already emitted inline above
            elif g % 2 == 1:
                flush_mv(i1 - 1)
            # pipelined ctx stages for batch b-1 (bf16-mode PE ops, placed
            # adjacent to the matvec block to avoid extra mode switches)
            if b > 0:
                if g == 0:
                    ctx_stage_bcast()
                elif g == 1:
                    ctx_stage_fused(enc_prev[2], enc_prev[1])
                elif g == 2:
                    ctx_stage_out(b - 1)

        if b < B_L - 1:
            # trailing matvec for i=7: strip 0 only needs tanh(7) chunk c0;
            # a single warm MM hides the deterministic ~270ns ScalarE
            # latency before chunk c1 (strips 1-2) lands
            th7 = th_of.pop(NT - 1)
            for k, (s, off, n) in enumerate(STRIPS):
                if k == 1:
                    warm(1)
                nc.tensor.matmul(
                    att_ps[32 * s : 32 * s + 1, 0:n],
                    v_sb[:, NT - 1 : NT],
                    th7[:, off : off + n],
                    start=False,
                    stop=True,
                    tile_position=(0, 32 * s),
                )

        # --- per-batch epilogue: exp straight from PSUM (no max needed; the
        # att bank is free long before the next batch's first matvec). The
        # softmax normalization happens on the HOST: the device ships raw
        # exp rows and only folds 1/Z into the context reductions. ---
        if b < B_L - 1:
            exp_stg = stgp.tile([1, S], F32, tag="expstg")
            ssum2 = stgp.tile([1, 3], F32, tag="ssum2")
            for s, off, n in STRIPS:
                nc.scalar.activation(
                    exp_stg[0:1, off : off + n],
                    att_ps[32 * s : 32 * s + 1, 0:n],
                    AF.Exp,
                    accum_out=ssum2[0:1, s : s + 1],
                )
            ssum = stgp.tile([1, 1], F32, tag="ssum")
            nc.vector.reduce_sum(ssum[:], ssum2[:], axis=mybir.AxisListType.X)
            srec = stgp.tile([1, 1], F32, tag="srec")
            nc.vector.reciprocal(srec[:], ssum[:])
            nc.gpsimd.dma_start(alpha_out[b : b + 1, :], exp_stg[0:1, :])
            # normalized bf16 alpha for the next batch's DVE context path
            nc.vector.tensor_scalar_mul(alpha_b16[0:1, :], exp_stg[0:1, :], srec[:])

            enc_prev = (enc8_cur, encbf_cur, encc_cur)
            enc8_cur, encbf_cur, encc_cur = enc_next
            continue

        # --- final batch: interleaved softmax + PE context chain ---
        # exp straight from PSUM in quarter chunks so the p-transposes can
        # start ~370ns after the last matvec; 1/Z folded into the evac
        exp_stg = stgp.tile([1, S], F32, tag="expstg")
        ssum4 = stgp.tile([1, 3], F32, tag="ssum4")
        for s, off, n in STRIPS:
            nc.scalar.activation(
                exp_stg[0:1, off : off + n],
                att_ps[32 * s : 32 * s + 1, 0:n],
                AF.Exp,
                accum_out=ssum4[0:1, s : s + 1],
            )
        warm(4)  # bridge the exp wait; keep the HAM gate open
        psk = psp.tile([P, NT], F32, tag="mm", bufs=5, name="psk")
        pT_sb = stgp.tile([P, NT], BF16, tag="pT")
        # two PSUM tiles (one per column strip): cross-engine readers of a
        # single PSUM tile serialize, and the evac runs on ScalarE + DVE
        ps3t = [
            psp.tile([P, 512], F32, tag="u", bufs=2, name=f"ps3{c}")
            for c in range(2)
        ]
        # all transposes first, grouped by which exp strip covers them, then
        # the context matvecs run back-to-back with no mid-chain exp wait
        for k0, k1 in ((0, 2), (2, 5), (5, 8)):
            for k in range(k0, k1):
                nc.tensor.transpose(
                    psk[:, k : k + 1],
                    exp_stg[0:1, k * P : (k + 1) * P],
                    identf[0:1, 0:1],
                )
            nc.vector.tensor_copy(pT_sb[:, k0:k1], psk[:, k0:k1])
        for k in range(NT):
            lhsT = pT_sb[:, k : k + 1]
            for c in range(2):
                nc.tensor.matmul(
                    ps3t[c][32 * c : 32 * c + 1, :],
                    lhsT,
                    encN3_sb[:, k, c * 512 : (c + 1) * 512],
                    start=(k == 0),
                    stop=(k == NT - 1),
                    tile_position=(0, 32 * c),
                )
        ssum = stgp.tile([1, 1], F32, tag="ssum")
        nc.vector.reduce_sum(ssum[:], ssum4[:], axis=mybir.AxisListType.X)
        srec = stgp.tile([1, 1], F32, tag="srec")
        nc.vector.reciprocal(srec[:], ssum[:])
        nc.scalar.dma_start(alpha_out[b : b + 1, :], exp_stg[0:1, :])
        # 1/Z-scaled PSUM evac into TWO tiles (a shared tile serializes the
        # cross-engine writers) with parallel DMAs on separate queues
        ctx3a = stgp.tile([1, 512], F32, tag="ctx3a")
        ctx3b = stgp.tile([1, 512], F32, tag="ctx3b")
        nc.scalar.activation(
            ctx3a[0:1, :], ps3t[0][0:1, :], AF.Copy, scale=srec[0:1, 0:1]
        )
        nc.vector.tensor_scalar_mul(ctx3b[0:1, :], ps3t[1][32:33, :], srec[:])
        nc.sync.dma_start(ctx_out[B_L - 1 : B_L, 0:512], ctx3a[0:1, :])
        nc.scalar.dma_start(ctx_out[B_L - 1 : B_L, 512:1024], ctx3b[0:1, :])
    ctx.close()


_CACHED = None


def _build():
    global _CACHED
    if _CACHED is None:
        nc = bacc.Bacc("TRN2", target_bir_lowering=False, debug=False)
        with tile.TileContext(nc) as tc:
            _emit(tc)
        nc.compile()
        _CACHED = nc
    return _CACHED


def make_in_maps(decoder_hidden, encoder_outputs, U_w, W_w, v_w):
    """Host-side layout prep: casts, pre-transposes, U_h projection."""
    dec = np.asarray(decoder_hidden, dtype=np.float32)
    enc = np.asarray(encoder_outputs, dtype=np.float32)
    U = np.asarray(U_w, dtype=np.float32)
    W = np.asarray(W_w, dtype=np.float32)
    v = np.asarray(v_w, dtype=np.float32)

    # U_h[b, o] = sum_h dec[b, h] U[o, h]  (exact f32, matches reference)
    U_h = dec @ U.T

    # W8[p, i, jp, i2, oo] = fp8(W[128i+oo, 256jp+128i2+p]), h in [0, 512)
    W8 = np.ascontiguousarray(
        W[:, : 128 * N8].reshape(NT, P, NP, 2, P).transpose(4, 0, 2, 3, 1).astype(E4)
    )
    # Wbf[p, jj, i, oo] = W[128i+oo, 128(N8+jj)+p]
    Wbf = np.ascontiguousarray(
        W[:, 128 * N8 :].reshape(NT, P, NBF, P).transpose(3, 2, 0, 1).astype(BF)
    )
    ident = np.eye(P, dtype=np.float32)
    vT = np.ascontiguousarray(v.reshape(NT, P).T.astype(BF))

    in_maps = []
    for c in range(N_CORES):
        sl = slice(c * B_L, (c + 1) * B_L)
        enc_sl = enc[sl]  # [B_L, S, H]
        enc_t = enc_sl.transpose(0, 2, 1)  # [b, h, s]
        # enc8[b, p, jp, i2, s] = fp8(enc[b, s, 256jp+128i2+p]), h in [0, 512)
        enc8 = np.ascontiguousarray(
            enc_t[:, : 128 * N8].reshape(B_L, NP, 2, P, S).transpose(0, 3, 1, 2, 4).astype(E4)
        )
        # encbf[b, p, jj, s] = enc[b, s, 128(N8+jj)+p]
        encbf = np.ascontiguousarray(
            enc_t[:, 128 * N8 :].reshape(B_L, NBF, P, S).transpose(0, 2, 1, 3).astype(BF)
        )
        # encc[b, p, j, s] = enc[b, s, 128j+p] for j < N8 (bf16 ctx copy)
        encc = np.ascontiguousarray(
            enc_t[:, : 128 * N8].reshape(B_L, N8, P, S).transpose(0, 2, 1, 3).astype(BF)
        )
        # encN3[p, k, h] = enc[B_L-1, 128k+p, h] (last batch, natural layout)
        encN3 = np.ascontiguousarray(
            enc_sl[B_L - 1].reshape(NT, P, H).transpose(1, 0, 2).astype(BF)
        )
        # UhT[p, j, b] = U_h[b, 128j+p]
        UhT = np.ascontiguousarray(
            U_h[sl].reshape(B_L, NT, P).transpose(2, 1, 0).astype(np.float32)
        )
        in_maps.append(
            {
                "enc8": enc8,
                "encbf": encbf,
                "encc": encc,
                "W8": W8,
                "Wbf": Wbf,
                "UhT": UhT,
                "vT": vT,
                "ident": ident,
                "encN3": encN3,
            }
        )
    return in_maps


def kernel(
    decoder_hidden: np.ndarray,
    encoder_outputs: np.ndarray,
    U_w: np.ndarray,
    W_w: np.ndarray,
    v_w: np.ndarray,
):
    nc = _build()
    in_maps = make_in_maps(decoder_hidden, encoder_outputs, U_w, W_w, v_w)
    res = run_bass_kernel_spmd(nc, in_maps, core_ids=list(range(N_CORES)))
    context = np.concatenate([res.results[c]["ctx"] for c in range(N_CORES)], axis=0)
    # the device ships unnormalized exp(att) rows; normalize here
    p = np.concatenate([res.results[c]["alpha"] for c in range(N_CORES)], axis=0)
    alpha = p / p.sum(axis=1, keepdims=True)
    return (context.astype(np.float32), alpha.astype(np.float32))
